# revision 3
# baseline (speedup 1.0000x reference)
"""Trainium2 Bass kernel for sliding-window (±64) multi-head attention.

Reference computation (seq=4096, hidden=768, 12 heads x 64, RoPE, window 128):
    qkv = qkv_weight @ x ; q,k = rope(q,k) ; scores = q^T k / 8 + band_mask
    attn = softmax(scores) @ v ; out = out_weight @ attn

Sharding: sequence-parallel over 8 cores. Core c owns queries
[512c, 512c+512) and computes K/V over the haloed span [512c-64, 512c+576)
(zero-padded at the sequence edges; padding is killed by the band mask).
Each core runs an identical Bass program on different data; the full output
is reassembled on host by concatenation (no collectives needed).

Engine notes: DVE/ACT lanes are partition-fixed, so rotate_half (a +-32
partition swap) is done as a PE matmul against a signed permutation matrix;
softmax normalization is folded into the P^T transpose matmul via diag(1/sum).
"""

import os
import sys

import numpy as np

for _p in ("/opt/trn_rl_repo",):
    if _p not in sys.path and os.path.isdir(_p):
        sys.path.insert(0, _p)

import ml_dtypes

import concourse.bass as bass
import concourse.bacc as bacc
import concourse.tile as tile
from concourse import mybir
from concourse.bass_utils import run_bass_kernel_spmd

F32 = mybir.dt.float32
F32R = mybir.dt.float32r
BF16 = mybir.dt.bfloat16

N_CORES = 8
SEQ = 4096
S_CORE = SEQ // N_CORES  # 512 queries per core
HALO = 64                # window // 2
SPAN = S_CORE + 2 * HALO  # 640 keys per core
HID = 768
NH = 12
DH = 64
NCH = HID // 128         # 6 contraction chunks
NHP = NH // 2            # 6 head pairs
NQB = S_CORE // 128      # 4 query blocks per core
NSC = SPAN // 128        # 5 key chunks per core
KSPAN = 256              # key span per query block

_BUILD_CACHE = {}


def _build(add_mask: bool):
    """Build + compile the per-core Bass program (shared by all 8 cores)."""
    nc = bacc.Bacc("TRN2", target_bir_lowering=False, debug=False, num_devices=N_CORES)

    xin = nc.dram_tensor("xin", [128, NCH * SPAN], F32R, kind="ExternalInput")
    wqt = nc.dram_tensor("wqt", [128, NCH * HID], F32R, kind="ExternalInput")
    wkt = nc.dram_tensor("wkt", [128, NCH * HID], F32R, kind="ExternalInput")
    wvt = nc.dram_tensor("wvt", [128, NCH * HID], F32R, kind="ExternalInput")
    wot = nc.dram_tensor("wot", [128, NCH * HID], F32R, kind="ExternalInput")
    cosb = nc.dram_tensor("cosb", [128, SPAN], F32, kind="ExternalInput")
    sinp = nc.dram_tensor("sinp", [128, SPAN], F32, kind="ExternalInput")
    perms = nc.dram_tensor("perms", [128, 128], F32R, kind="ExternalInput")
    maskb = nc.dram_tensor("maskb", [128, NQB * KSPAN], BF16, kind="ExternalInput")
    if add_mask:
        maskf = nc.dram_tensor("maskf", [128, NQB * KSPAN], F32, kind="ExternalInput")
    diag = nc.dram_tensor("diag", [128, 128], BF16, kind="ExternalInput")
    out_d = nc.dram_tensor("out", [128, NCH * S_CORE], F32, kind="ExternalOutput")

    with tile.TileContext(nc) as tc:
        from contextlib import ExitStack

        with ExitStack() as ctx:
            const = ctx.enter_context(tc.tile_pool(name="const", bufs=1))
            sb = ctx.enter_context(tc.tile_pool(name="sb", bufs=1))
            tmp = ctx.enter_context(tc.tile_pool(name="tmp", bufs=3))
            attnp = ctx.enter_context(tc.tile_pool(name="attnp", bufs=4))
            scal = ctx.enter_context(tc.tile_pool(name="scal", bufs=6))
            outp = ctx.enter_context(tc.tile_pool(name="outp", bufs=2))
            ps_proj = ctx.enter_context(
                tc.tile_pool(name="ps_proj", bufs=2, space="PSUM")
            )
            ps_att = ctx.enter_context(
                tc.tile_pool(name="ps_att", bufs=4, space="PSUM")
            )
            ps_o = ctx.enter_context(tc.tile_pool(name="ps_o", bufs=2, space="PSUM"))

            # ---- load inputs to SBUF ----
            X = sb.tile([128, NCH * SPAN], F32R, tag="X")
            nc.sync.dma_start(out=X[:], in_=xin.ap())
            WQT = sb.tile([128, NCH * HID], F32R, tag="WQT")
            nc.sync.dma_start(out=WQT[:], in_=wqt.ap())
            COS = const.tile([128, SPAN], F32, tag="COS")
            nc.sync.dma_start(out=COS[:], in_=cosb.ap())
            SINP = const.tile([128, SPAN], F32, tag="SINP")
            nc.sync.dma_start(out=SINP[:], in_=sinp.ap())
            PERMS = const.tile([128, 128], F32R, tag="PERMS")
            nc.sync.dma_start(out=PERMS[:], in_=perms.ap())
            WKT = sb.tile([128, NCH * HID], F32R, tag="WKT")
            nc.sync.dma_start(out=WKT[:], in_=wkt.ap())
            WVT = sb.tile([128, NCH * HID], F32R, tag="WVT")
            nc.sync.dma_start(out=WVT[:], in_=wvt.ap())
            MB = const.tile([128, NQB * KSPAN], BF16, tag="MB")
            nc.sync.dma_start(out=MB[:], in_=maskb.ap())
            if add_mask:
                MF = const.tile([128, NQB * KSPAN], F32, tag="MF")
                nc.sync.dma_start(out=MF[:], in_=maskf.ap())
            DIAG = const.tile([128, 128], BF16, tag="DIAG")
            nc.sync.dma_start(out=DIAG[:], in_=diag.ap())
            WOT = sb.tile([128, NCH * HID], F32R, tag="WOT")
            nc.sync.dma_start(out=WOT[:], in_=wot.ap())

            # persistent intermediates
            Qs = sb.tile([128, NHP * S_CORE], F32R, tag="Qs")   # [2hd, (hp, s)]
            Ks = sb.tile([128, NHP * SPAN], F32R, tag="Ks")     # [2hd, (hp, s)]
            VT = sb.tile([128, NSC * HID], BF16, tag="VT")     # [s, (chunk, hd)]
            AT = sb.tile([128, NCH * S_CORE], F32R, tag="AT")   # [c, (cchunk, s)]

            mult = mybir.AluOpType.mult
            addop = mybir.AluOpType.add

            def rope(dst, src_ps, cos_ap, sin_ap, w):
                # dst = src*cos + rot(src)*sin ; rot via PE permutation matmul
                qsb = tmp.tile([128, S_CORE], F32R, tag="ropet")
                nc.scalar.copy(qsb[:, :w], src_ps)
                qrot = ps_att.tile([128, w], F32, tag="att")
                nc.tensor.matmul(
                    qrot[:], PERMS[:], qsb[:, :w],
                    start=True, stop=True,
                )
                nc.vector.tensor_tensor(dst, qsb[:, :w], cos_ap, op=mult)
                m2 = tmp.tile([128, S_CORE], F32, tag="ropem")
                nc.vector.tensor_tensor(m2[:, :w], qrot[:], sin_ap, op=mult)
                nc.vector.tensor_tensor(dst, dst, m2[:, :w], op=addop)

            # ---- Q projection (+RoPE): Q[2hd, s] per head pair ----
            for t in range(NHP):
                qp = ps_proj.tile([128, S_CORE], F32, tag="proj")
                for k in range(NCH):
                    nc.tensor.matmul(
                        qp[:],
                        WQT[:, k * HID + t * 128 : k * HID + (t + 1) * 128],
                        X[:, k * SPAN + HALO : k * SPAN + HALO + S_CORE],
                        start=(k == 0),
                        stop=(k == NCH - 1),
                    )
                rope(
                    Qs[:, t * S_CORE : (t + 1) * S_CORE],
                    qp[:],
                    COS[:, HALO : HALO + S_CORE],
                    SINP[:, HALO : HALO + S_CORE],
                    S_CORE,
                )

            # ---- K projection (+RoPE) over the haloed span ----
            for t in range(NHP):
                for half in range(2):
                    w = SPAN // 2  # 320
                    kp = ps_proj.tile([128, w], F32, tag="proj")
                    for k in range(NCH):
                        nc.tensor.matmul(
                            kp[:],
                            WKT[:, k * HID + t * 128 : k * HID + (t + 1) * 128],
                            X[:, k * SPAN + half * w : k * SPAN + (half + 1) * w],
                            start=(k == 0),
                            stop=(k == NCH - 1),
                        )
                    rope(
                        Ks[:, t * SPAN + half * w : t * SPAN + (half + 1) * w],
                        kp[:],
                        COS[:, half * w : (half + 1) * w],
                        SINP[:, half * w : (half + 1) * w],
                        w,
                    )

            # ---- V^T projection: VT[s, hd] per 128-key chunk (bf16) ----
            for sc in range(NSC):
                for hf in range(2):
                    w = HID // 2  # 384
                    vp = ps_proj.tile([128, w], F32, tag="proj")
                    for k in range(NCH):
                        nc.tensor.matmul(
                            vp[:],
                            X[:, k * SPAN + sc * 128 : k * SPAN + (sc + 1) * 128],
                            WVT[:, k * HID + hf * w : k * HID + (hf + 1) * w],
                            start=(k == 0),
                            stop=(k == NCH - 1),
                        )
                    nc.scalar.copy(VT[:, sc * HID + hf * w : sc * HID + (hf + 1) * w], vp[:])

            # ---- attention: per (query block, head) ----
            exp = mybir.ActivationFunctionType.Exp
            for qb in range(NQB):
                for hp in range(NHP):
                    op_ = ps_o.tile([128, 128], F32, tag="o")
                    for h in range(2):
                        hg = hp * 2 + h
                        sp = ps_att.tile([128, KSPAN], F32, tag="att")
                        nc.tensor.matmul(
                            sp[:],
                            Qs[64 * h : 64 * (h + 1),
                               hp * S_CORE + qb * 128 : hp * S_CORE + (qb + 1) * 128],
                            Ks[64 * h : 64 * (h + 1),
                               hp * SPAN + qb * 128 : hp * SPAN + qb * 128 + KSPAN],
                            start=True,
                            stop=True,
                        )
                        praw = attnp.tile([128, KSPAN], BF16, tag="praw")
                        if add_mask:
                            ssb = tmp.tile([128, KSPAN], F32, tag="ssb")
                            nc.vector.tensor_tensor(
                                ssb[:], sp[:],
                                MF[:, qb * KSPAN : (qb + 1) * KSPAN], op=addop,
                            )
                            nc.scalar.activation(praw[:], ssb[:], exp)
                        else:
                            nc.scalar.activation(praw[:], sp[:], exp)
                        P = attnp.tile([128, KSPAN], BF16, tag="P")
                        nc.vector.tensor_tensor(
                            P[:], praw[:], MB[:, qb * KSPAN : (qb + 1) * KSPAN], op=mult
                        )
                        ssum = scal.tile([128, 1], F32, tag="ssum")
                        nc.vector.reduce_sum(ssum[:], P[:], axis=mybir.AxisListType.X)
                        rr = scal.tile([128, 1], F32, tag="rr")
                        nc.vector.reciprocal(rr[:], ssum[:])
                        dr = attnp.tile([128, 128], BF16, tag="dr")
                        nc.vector.tensor_scalar_mul(dr[:], DIAG[:], rr[:])
                        # P^T (and normalize): pt[:, q] = P^T @ diag(r)
                        pt = ps_att.tile([128, KSPAN], F32, tag="att")
                        nc.tensor.matmul(pt[:, 0:128], P[:, 0:128], dr[:],
                                         start=True, stop=True)
                        nc.tensor.matmul(pt[:, 128:256], P[:, 128:256], dr[:],
                                         start=True, stop=True)
                        pts = attnp.tile([128, KSPAN], BF16, tag="pts")
                        nc.scalar.copy(pts[:, 0:128], pt[:, 0:128])
                        nc.vector.tensor_copy(pts[:, 128:256], pt[:, 128:256])
                        # attn[d, q] += V^T[j, d]^T @ P^T[j, q]
                        # head h lands on PSUM partitions 64h..64h+64 (col group)
                        osl = op_[64 * h : 64 * (h + 1), :]
                        tp = (0, 64 * h)
                        nc.tensor.matmul(
                            osl, VT[:, qb * HID + hg * 64 : qb * HID + hg * 64 + 64],
                            pts[:, 0:128], start=True, stop=False, tile_position=tp,
                        )
                        nc.tensor.matmul(
                            osl,
                            VT[:, (qb + 1) * HID + hg * 64 : (qb + 1) * HID + hg * 64 + 64],
                            pts[:, 128:256], start=False, stop=True, tile_position=tp,
                        )
                    nc.vector.tensor_copy(
                        AT[:, hp * S_CORE + qb * 128 : hp * S_CORE + (qb + 1) * 128],
                        op_[:],
                    )

            # ---- output projection ----
            for oc in range(NCH):
                ops = ps_proj.tile([128, S_CORE], F32, tag="proj")
                for k in range(NCH):
                    nc.tensor.matmul(
                        ops[:],
                        WOT[:, k * HID + oc * 128 : k * HID + (oc + 1) * 128],
                        AT[:, k * S_CORE : (k + 1) * S_CORE],
                        start=(k == 0),
                        stop=(k == NCH - 1),
                    )
                ot = outp.tile([128, S_CORE], F32, tag="ot")
                nc.scalar.copy(ot[:], ops[:])
                nc.sync.dma_start(
                    out=out_d.ap()[:, oc * S_CORE : (oc + 1) * S_CORE], in_=ot[:]
                )

    nc.compile()
    return nc


def get_program(add_mask: bool):
    if add_mask not in _BUILD_CACHE:
        _BUILD_CACHE[add_mask] = _build(add_mask)
    return _BUILD_CACHE[add_mask]


def _pack_chunked(a, nch, w):
    """[nch*128, w] row-major -> [128, nch*w] with chunk-major free dim."""
    return np.ascontiguousarray(
        a.reshape(nch, 128, w).transpose(1, 0, 2).reshape(128, nch * w)
    )


def prep_core_inputs(core, xs, pos, am, qkv_weight, out_weight, add_mask):
    """Build the per-core input map (numpy) for one core."""
    start = S_CORE * core - HALO
    idx = np.arange(start, start + SPAN)
    valid = (idx >= 0) & (idx < SEQ)

    Xs = np.zeros((HID, SPAN), np.float32)
    Xs[:, valid] = xs[:, idx[valid]]

    pspan = np.zeros((SPAN,), np.float32)
    pspan[valid] = pos[idx[valid]]
    invf = (
        1.0 / (10000.0 ** (np.arange(0, DH, 2, dtype=np.float32) / np.float32(DH)))
    ).astype(np.float32)
    f = pspan[None, :] * invf[:, None]  # [32, SPAN]
    cos32 = np.cos(f).astype(np.float32)
    sin32 = np.sin(f).astype(np.float32)
    COS = np.tile(cos32, (4, 1))
    SINP = np.tile(sin32, (4, 1))

    # signed rotate-half permutation: (PERMS.T @ q)[d] = rot_half(q)[d]
    di = np.arange(128)
    lo = (di % 64) < 32
    src = np.where(lo, di + 32, di - 32)
    sgn = np.where(lo, -1.0, 1.0).astype(np.float32)
    PERMS = np.zeros((128, 128), np.float32)
    PERMS[src, di] = sgn

    mb = np.zeros((128, NQB, KSPAN), np.float32)
    mf = np.full((128, NQB, KSPAN), -10000.0, np.float32)
    for qb in range(NQB):
        qg = S_CORE * core + 128 * qb + np.arange(128)
        kg = S_CORE * core + 128 * qb - HALO + np.arange(KSPAN)
        kvalid = (kg >= 0) & (kg < SEQ)
        band = (np.abs(kg[None, :] - qg[:, None]) <= HALO) & kvalid[None, :]
        mb[:, qb, :] = band
        if add_mask:
            amband = np.zeros((128, KSPAN), np.float32)
            amband[:, kvalid] = am[np.ix_(qg, kg[kvalid])]
            mf[:, qb, :] = np.where(band, amband, -10000.0)

    wq = qkv_weight[0:HID] * np.float32(DH**-0.5)
    wk = qkv_weight[HID : 2 * HID]
    wv = qkv_weight[2 * HID : 3 * HID]

    def packw(w):
        return _pack_chunked(np.ascontiguousarray(w.T.astype(np.float32)), NCH, HID)

    in_map = {
        "xin": _pack_chunked(Xs, NCH, SPAN),
        "wqt": packw(wq),
        "wkt": packw(wk),
        "wvt": packw(wv),
        "wot": packw(out_weight),
        "cosb": COS,
        "sinp": SINP,
        "perms": PERMS,
        "maskb": mb.reshape(128, NQB * KSPAN).astype(ml_dtypes.bfloat16),
        "diag": np.eye(128, dtype=ml_dtypes.bfloat16),
    }
    if add_mask:
        in_map["maskf"] = np.ascontiguousarray(mf.reshape(128, NQB * KSPAN))
    return in_map


def prep_all_inputs(x, position_ids, attention_mask, qkv_weight, out_weight):
    xs = np.asarray(x, dtype=np.float32)[0, :, 0, :]  # [768, 4096]
    pos = np.asarray(position_ids)[0].astype(np.float32)
    am = np.asarray(attention_mask, dtype=np.float32)[0, 0]
    qkv_w = np.asarray(qkv_weight, dtype=np.float32)
    out_w = np.asarray(out_weight, dtype=np.float32)
    add_mask = bool(np.any(am))
    in_maps = [
        prep_core_inputs(c, xs, pos, am, qkv_w, out_w, add_mask)
        for c in range(N_CORES)
    ]
    return in_maps, add_mask


def assemble_output(results):
    cols = []
    for c in range(N_CORES):
        o = np.asarray(results[c]["out"])  # [128, 6*512]
        cols.append(o.reshape(128, NCH, S_CORE).transpose(1, 0, 2).reshape(HID, S_CORE))
    full = np.concatenate(cols, axis=1)  # [768, 4096]
    return np.ascontiguousarray(full.reshape(1, HID, 1, SEQ), dtype=np.float32)


def kernel(**inputs):
    in_maps, add_mask = prep_all_inputs(
        inputs["x"],
        inputs["position_ids"],
        inputs["attention_mask"],
        inputs["qkv_weight"],
        inputs["out_weight"],
    )
    nc = get_program(add_mask)
    res = run_bass_kernel_spmd(nc, in_maps, core_ids=list(range(N_CORES)))
    return assemble_output(res.results)
